# revision 1
# baseline (speedup 1.0000x reference)
"""GQA (B=1, S=2048, D=4096, H=32, G=8) on 8 TRN2 NeuronCores.

Sharding: tensor-parallel over heads — core c owns query heads 4c..4c+3 and
KV group c. Per core: qT/kT/vT projections from full x (transposed layouts),
RoPE, causal attention with transposed probs (exp without max-subtraction —
scores are bounded; denominator via a ones-column in V'), normalization folded
into a per-partition scalar multiply, PE-transpose of ctx, AllGather of ctxT
(4MB/rank), then a column-sharded Wo matmul. Host slices weights / transposes
x / concatenates output columns.

Self-contained: no sibling imports; hardcoded shapes.
"""
import contextlib
import ctypes
import os
import sys
import types

import numpy as np

os.environ.setdefault("MYCRO_LOCAL_CACHE", "1")

for _p in ("/opt/trn_rl_repo", "/root/.axon_site/_ro/trn_rl_repo"):
    if _p not in sys.path and os.path.isdir(_p):
        sys.path.append(_p)

import concourse.bass as bass
import concourse.tile as tile
from concourse import mybir
from concourse.bass_utils import run_bass_kernel_spmd
from concourse.masks import make_identity

# ---------------------------------------------------------------- profiling shim
_SO_PATH = "/opt/axon/libaxon_pjrt.so"
_hook_holder = [None]


def _ntff_profile_via_ctypes(so_path):
    try:
        lib = ctypes.CDLL(so_path)
    except OSError:
        return None
    if not hasattr(lib, "axon_start_nrt_profile"):
        return None
    lib.axon_start_nrt_profile.argtypes = [
        ctypes.POINTER(ctypes.c_int64),
        ctypes.c_size_t,
    ]
    lib.axon_start_nrt_profile.restype = ctypes.c_int64
    lib.axon_stop_nrt_profile.argtypes = [ctypes.c_char_p]
    lib.axon_stop_nrt_profile.restype = ctypes.c_int64

    @contextlib.contextmanager
    def _hook(output_dir, device_ids):
        import jax

        jax.devices()
        if device_ids:
            ids = (ctypes.c_int64 * len(device_ids))(*device_ids)
            rc = lib.axon_start_nrt_profile(ids, len(device_ids))
        else:
            rc = lib.axon_start_nrt_profile(None, 0)
        if rc != 0:
            raise RuntimeError(f"axon_start_nrt_profile rc={rc}")
        try:
            yield
        finally:
            n = lib.axon_stop_nrt_profile(str(output_dir).encode())
            if n <= 0:
                print(f"WARNING: ntff capture wrote {n} files", file=sys.stderr)

    return _hook


def _install_prof_shim():
    if "antenv.axon_hooks" not in sys.modules:
        mod = types.ModuleType("antenv.axon_hooks")
        mod.set_axon_ntff_profile_hook = lambda h: _hook_holder.__setitem__(0, h)
        mod.get_axon_ntff_profile_hook = lambda: _hook_holder[0]
        sys.modules["antenv.axon_hooks"] = mod
    _hook_holder[0] = _ntff_profile_via_ctypes(_SO_PATH)
    import concourse.bass_utils as bu

    bu.upload_artifacts = lambda tmpdir: tmpdir


_install_prof_shim()

# ------------------------------------------------------------- wait-split pass
def _split_multi_waits(nc, maxw=1):
    """walrus in this container allows only one sync-wait per instruction;
    split extras onto nops inserted before the offender (same engine/block)."""

    def _remove_by_name(name):
        for f in nc.m.functions:
            for bb in f.blocks:
                for i, inst in enumerate(bb.instructions):
                    if inst.name == name:
                        lst = bb.instructions
                        del lst[i]
                        bb.instructions = lst
                        return inst
        raise KeyError(name)

    offenders = []
    for f in nc.m.functions:
        for bb in f.blocks:
            for inst in bb.instructions:
                si = inst.sync_info
                if si and si.on_wait and len(si.on_wait) > maxw:
                    offenders.append(inst.name)
    for name in offenders:
        target = None
        for f in nc.m.functions:
            for bb in f.blocks:
                for idx, inst in enumerate(bb.instructions):
                    if inst.name == name:
                        target = (bb, inst)
                        break
                if target:
                    break
            if target:
                break
        bb, inst = target
        waits = list(inst.sync_info.on_wait)
        updates = list(inst.sync_info.on_update or [])
        chunks = [waits[i:i + maxw] for i in range(0, len(waits), maxw)]
        nops = []
        for ch in chunks[:-1]:
            bnop = nc.engines[inst.engine].nop(nofuse=True, hint="waitsplit")
            nop_inst = _remove_by_name(bnop.ins.name)
            nop_inst.sync_info = mybir.SyncInfo(on_wait=ch, on_update=[])
            nops.append(nop_inst)
        inst.sync_info = mybir.SyncInfo(on_wait=chunks[-1], on_update=updates)
        lst = bb.instructions
        idx = next(i for i, x in enumerate(lst) if x.name == name)
        lst[idx:idx] = nops
        bb.instructions = lst
    return len(offenders)


# ------------------------------------------------------------------- constants
B, S, D = 1, 2048, 4096
H, G = 32, 8
HD = D // H            # 128
NC = 8                 # cores
HPC = H // NC          # heads per core = 4
OC = D // NC           # out columns per core = 512
P = 128
KT = D // P            # 32 contraction tiles
SCH = 256              # sequence chunk width for projections/attention
NSC = S // SCH         # 8
NKB = S // P           # 16 key tiles
SCALE = float(1.0 / np.sqrt(np.float32(HD)))

f32 = mybir.dt.float32
f32r = mybir.dt.float32r
bf16 = mybir.dt.bfloat16

Copy = mybir.ActivationFunctionType.Copy
Exp = mybir.ActivationFunctionType.Exp


def _build_program():
    nc = bass.Bass()
    xT = nc.declare_dram_parameter("xT", [P, KT, S], f32r, isOutput=False)
    wq = nc.declare_dram_parameter("wq", [P, KT, OC], f32r, isOutput=False)
    wk = nc.declare_dram_parameter("wk", [P, KT, HD], f32r, isOutput=False)
    wv = nc.declare_dram_parameter("wv", [P, KT, HD], f32r, isOutput=False)
    wo = nc.declare_dram_parameter("wo", [P, KT, OC], f32r, isOutput=False)
    cosT = nc.declare_dram_parameter("cosT", [HD, S], f32, isOutput=False)
    sinT = nc.declare_dram_parameter("sinT", [HD, S], f32, isOutput=False)
    tri = nc.declare_dram_parameter("tri", [P, P], f32, isOutput=False)
    out = nc.declare_dram_parameter("out", [S, OC], f32, isOutput=True)

    NSPL = 4
    SPW = S // NSPL  # 512 columns per collective split
    cc_ins = [nc.dram_tensor(f"cc_in{k}", [HPC * HD, SPW], f32r)
              for k in range(NSPL)]
    cc_outs = [nc.dram_tensor(f"cc_out{k}", [D, SPW], f32r, addr_space="Shared")
               for k in range(NSPL)]

    with tile.TileContext(nc) as tc:
        with (
            tc.tile_pool(name="singles", bufs=1) as singles,
            tc.tile_pool(name="wbig", bufs=1) as wbig,
            tc.tile_pool(name="stream", bufs=6) as stream,
            tc.tile_pool(name="qts", bufs=6) as qtsp,
            tc.tile_pool(name="pt", bufs=17) as ptp,
            tc.tile_pool(name="work", bufs=6) as work,
            tc.tile_pool(name="evict", bufs=4) as evictp,
            tc.tile_pool(name="ps", bufs=1, space="PSUM") as psp,
        ):
            # ---- constants / persistents
            ident = singles.tile([P, P], f32)
            make_identity(nc, ident[:])
            trim = singles.tile([P, P], bf16)
            tri_sb = singles.tile([P, P], f32)
            nc.sync.dma_start(out=tri_sb[:], in_=tri[:])
            nc.vector.tensor_copy(trim[:], tri_sb[:])

            cos_sb = singles.tile([HD, S], f32)
            nc.sync.dma_start(out=cos_sb[:], in_=cosT[:])
            sin_sb = singles.tile([HD, S], f32)
            nc.sync.dma_start(out=sin_sb[:], in_=sinT[:])

            wq_sb = wbig.tile([P, KT, OC], f32r, tag="wbig")
            nc.sync.dma_start(out=wq_sb[:], in_=wq[:])
            wk_sb = singles.tile([P, KT, HD], f32r)
            nc.sync.dma_start(out=wk_sb[:], in_=wk[:])
            wv_sb = singles.tile([P, KT, HD], f32r)
            nc.sync.dma_start(out=wv_sb[:], in_=wv[:])

            kT_all = singles.tile([HD, S], f32r)
            vp_all = singles.tile([P, NKB, HD + 1], bf16)
            nc.vector.memset(vp_all[:], 1.0)

            def rope_evict(ps_t, dst, dst0, tab0):
                """ps_t: PSUM [HD, SCH] pre-rope; writes dst[:, dst0:dst0+SCH]
                (f32r) using rope tables at absolute position tab0."""
                rot = work.tile([HD, SCH], f32, tag="rot", bufs=2)
                nc.scalar.activation(out=rot[0:64, :], in_=ps_t[64:128, :],
                                     func=Copy, scale=-1.0)
                nc.scalar.activation(out=rot[64:128, :], in_=ps_t[0:64, :],
                                     func=Copy)
                m1 = work.tile([HD, SCH], f32, tag="m1", bufs=2)
                nc.vector.tensor_mul(m1[:], ps_t[:], cos_sb[:, tab0:tab0 + SCH])
                nc.vector.tensor_mul(rot[:], rot[:], sin_sb[:, tab0:tab0 + SCH])
                nc.vector.tensor_add(dst[:, dst0:dst0 + SCH], m1[:], rot[:])

            # ================= phase 1: projections + attention, per s-chunk
            for sc in range(NSC):
                s0 = sc * SCH
                xtg = []
                for g in range(KT // 8):
                    t = stream.tile([P, 8, SCH], f32r, tag="stream", bufs=6)
                    nc.sync.dma_start(out=t[:],
                                      in_=xT[:, 8 * g:8 * g + 8, s0:s0 + SCH])
                    xtg.append(t)

                def xts(kt):
                    return xtg[kt // 8][:, kt % 8, :]

                # K projection -> RoPE -> kT_all
                ps_k = psp.tile([P, SCH], f32, tag="a", bufs=3)
                for kt in range(KT):
                    nc.tensor.matmul(ps_k[:], wk_sb[:, kt, :], xts(kt),
                                     start=(kt == 0), stop=(kt == KT - 1))
                rope_evict(ps_k, kT_all, s0, s0)

                # V projection (transposed) -> vT_all
                ps_v = psp.tile([P, SCH], f32, tag="a", bufs=3)
                for kt in range(KT):
                    nc.tensor.matmul(ps_v[:], wv_sb[:, kt, :], xts(kt),
                                     start=(kt == 0), stop=(kt == KT - 1))
                vc = work.tile([HD, SCH], f32, tag="vc", bufs=2)
                nc.scalar.copy(vc[:], ps_v[:])
                for half in range(SCH // P):
                    kb = sc * (SCH // P) + half
                    ps_vt = psp.tile([P, P], f32, tag="b", bufs=2)
                    nc.tensor.transpose(
                        ps_vt[:], vc[:, half * P:(half + 1) * P], ident[:]
                    )
                    nc.scalar.copy(vp_all[:, kb, 0:HD], ps_vt[:])

                # Q projections + RoPE (4 heads)
                qts = []
                for h in range(HPC):
                    ps_q = psp.tile([P, SCH], f32, tag="a", bufs=3)
                    for kt in range(KT):
                        nc.tensor.matmul(
                            ps_q[:], wq_sb[:, kt, h * P:(h + 1) * P], xts(kt),
                            start=(kt == 0), stop=(kt == KT - 1))
                    qt = qtsp.tile([HD, SCH], f32r, tag="qts")
                    rope_evict(ps_q, qt, 0, s0)
                    qts.append(qt)

                # attention for this chunk's queries
                nq = SCH // P  # q-subchunks of 128
                for h in range(HPC):
                    pts = []
                    for kb in range(nq * sc + nq):
                        ps_s = psp.tile([P, SCH], f32, tag="a", bufs=3)
                        nc.tensor.matmul(ps_s[:], kT_all[:, kb * P:(kb + 1) * P],
                                         qts[h][:], start=True, stop=True)
                        pt = ptp.tile([P, SCH], bf16, tag="pt")
                        nc.scalar.activation(out=pt[:], in_=ps_s[:],
                                             func=Exp, scale=SCALE)
                        diag = kb - nq * sc
                        if 0 <= diag < nq:
                            nc.vector.tensor_mul(
                                pt[:, diag * P:(diag + 1) * P],
                                pt[:, diag * P:(diag + 1) * P], trim[:])
                        pts.append(pt)

                    for qh in range(nq):
                        iqc = nq * sc + qh
                        ps_c = psp.tile([P, HD + 1], f32, tag="b", bufs=2)
                        for kb in range(iqc + 1):
                            nc.tensor.matmul(
                                ps_c[:], pts[kb][:, qh * P:(qh + 1) * P],
                                vp_all[:, kb, :],
                                start=(kb == 0), stop=(kb == iqc))
                        rden = work.tile([P, 1], f32, tag="rden", bufs=4)
                        nc.vector.reciprocal(rden[:], ps_c[:, HD:HD + 1])
                        ctxn = work.tile([P, HD], f32, tag="ctxn", bufs=4)
                        nc.vector.tensor_scalar_mul(ctxn[:], ps_c[:, 0:HD],
                                                    rden[:])
                        ps_t = psp.tile([P, P], f32, tag="b", bufs=2)
                        nc.tensor.transpose(ps_t[:], ctxn[:], ident[:])
                        ctxT_sb = evictp.tile([HD, P], f32r, tag="ctxT", bufs=3)
                        nc.scalar.copy(ctxT_sb[:], ps_t[:])
                        spl, lc = iqc // 4, iqc % 4
                        nc.sync.dma_start(
                            out=cc_ins[spl][h * HD:(h + 1) * HD,
                                            lc * P:(lc + 1) * P],
                            in_=ctxT_sb[:])

                if sc % 2 == 1:
                    k = (sc - 1) // 2
                    nc.gpsimd.collective_compute(
                        "AllGather",
                        mybir.AluOpType.bypass,
                        replica_groups=[list(range(NC))],
                        ins=[cc_ins[k][:]],
                        outs=[cc_outs[k][:]],
                    )

            # ================= phase 2: Wo (collectives were issued in-loop)
            wo_sb = wbig.tile([P, KT, OC], f32r, tag="wbig")
            nc.sync.dma_start(out=wo_sb[:], in_=wo[:])

            cc3s = [cc_outs[k][:].rearrange("(t p) s -> p t s", p=P)
                    for k in range(NSPL)]
            for sq in range(S // P):
                spl, ls = sq // 4, sq % 4
                ccts = []
                for g in range(2):
                    t = stream.tile([P, 16, P], f32r, tag="stream", bufs=6)
                    nc.sync.dma_start(
                        out=t[:],
                        in_=cc3s[spl][:, 16 * g:16 * g + 16,
                                      ls * P:(ls + 1) * P])
                    ccts.append(t)
                ps_o = psp.tile([P, OC], f32, tag="c", bufs=1)
                for kt in range(KT):
                    nc.tensor.matmul(
                        ps_o[:], ccts[kt // 16][:, kt % 16, :], wo_sb[:, kt, :],
                        start=(kt == 0), stop=(kt == KT - 1))
                out_sb = evictp.tile([P, OC], f32, tag="osb", bufs=2)
                nc.scalar.copy(out_sb[:], ps_o[:])
                nc.sync.dma_start(out=out[sq * P:(sq + 1) * P, :], in_=out_sb[:])

    return nc


_PROGRAM_CACHE = {}


def _get_program():
    if "nc" not in _PROGRAM_CACHE:
        nc = _build_program()
        _split_multi_waits(nc, maxw=1)
        _PROGRAM_CACHE["nc"] = nc
    return _PROGRAM_CACHE["nc"]


def _rope_tables_T():
    inv_freq = (1.0 / (10000.0 ** (np.arange(0, HD, 2, dtype=np.float32) / HD))
                ).astype(np.float32)
    ang = np.arange(S, dtype=np.float32)[:, None] * inv_freq[None, :]
    ang = np.concatenate([ang, ang], axis=-1)  # [S, HD]
    return (np.ascontiguousarray(np.cos(ang).T.astype(np.float32)),
            np.ascontiguousarray(np.sin(ang).T.astype(np.float32)))


def _prep_in_maps(x, Wq, Wk, Wv, Wo):
    x2d = np.asarray(x, np.float32).reshape(S, D)
    xT_dev = np.ascontiguousarray(x2d.T.reshape(KT, P, S).transpose(1, 0, 2))
    cosT, sinT = _rope_tables_T()
    tri_np = (np.arange(P)[:, None] <= np.arange(P)[None, :]).astype(np.float32)

    def wtiles(Wslice, width):
        return np.ascontiguousarray(
            np.asarray(Wslice, np.float32).reshape(KT, P, width)
            .transpose(1, 0, 2))

    in_maps = []
    for c in range(NC):
        in_maps.append({
            "xT": xT_dev,
            "wq": wtiles(Wq[:, c * OC:(c + 1) * OC], OC),
            "wk": wtiles(Wk[:, c * HD:(c + 1) * HD], HD),
            "wv": wtiles(Wv[:, c * HD:(c + 1) * HD], HD),
            "wo": wtiles(Wo[:, c * OC:(c + 1) * OC], OC),
            "cosT": cosT,
            "sinT": sinT,
            "tri": tri_np,
        })
    return in_maps


def _run(inputs, trace=False):
    nc = _get_program()
    in_maps = _prep_in_maps(inputs["x"], inputs["Wq"], inputs["Wk"],
                            inputs["Wv"], inputs["Wo"])
    res = run_bass_kernel_spmd(nc, in_maps, core_ids=list(range(NC)),
                               trace=trace)
    out = np.concatenate([res.results[c]["out"] for c in range(NC)], axis=1)
    return out.reshape(B, S, D).astype(np.float32), res


def kernel(**inputs):
    out, _ = _run(inputs, trace=False)
    return out



# revision 6
# speedup vs baseline: 1.2302x; 1.2302x over previous
"""GQA (B=1, S=2048, D=4096, H=32, G=8) on 8 TRN2 NeuronCores.

Tensor-parallel over heads: core c owns query heads 4c..4c+3 and KV group c.
v2: all-bf16 datapath (weights/x/k/q/probs/ctx/Wo and the AllGather wire),
SCH=512 sequence chunks, per-head-pair AllGathers issued mid-chunk, Wo
matmuls interleaved into the attention chunks as tensor-engine filler, and
output produced transposed ([OC, S]) so Wo keeps its weights stationary.
Host slices weights / transposes x / re-transposes the output.

Self-contained: no sibling imports; hardcoded shapes.
"""
import contextlib
import ctypes
import math
import os
import sys
import types

import ml_dtypes
import numpy as np

os.environ.setdefault("MYCRO_LOCAL_CACHE", "1")

for _p in ("/opt/trn_rl_repo", "/root/.axon_site/_ro/trn_rl_repo"):
    if _p not in sys.path and os.path.isdir(_p):
        sys.path.append(_p)

import concourse.bass as bass
import concourse.tile as tile
from concourse import mybir
from concourse.bass_utils import run_bass_kernel_spmd
from concourse.masks import make_identity

# ---------------------------------------------------------------- profiling shim
_SO_PATH = "/opt/axon/libaxon_pjrt.so"
_hook_holder = [None]


def _ntff_profile_via_ctypes(so_path):
    try:
        lib = ctypes.CDLL(so_path)
    except OSError:
        return None
    if not hasattr(lib, "axon_start_nrt_profile"):
        return None
    lib.axon_start_nrt_profile.argtypes = [
        ctypes.POINTER(ctypes.c_int64),
        ctypes.c_size_t,
    ]
    lib.axon_start_nrt_profile.restype = ctypes.c_int64
    lib.axon_stop_nrt_profile.argtypes = [ctypes.c_char_p]
    lib.axon_stop_nrt_profile.restype = ctypes.c_int64

    @contextlib.contextmanager
    def _hook(output_dir, device_ids):
        import jax

        jax.devices()
        if device_ids:
            ids = (ctypes.c_int64 * len(device_ids))(*device_ids)
            rc = lib.axon_start_nrt_profile(ids, len(device_ids))
        else:
            rc = lib.axon_start_nrt_profile(None, 0)
        if rc != 0:
            raise RuntimeError(f"axon_start_nrt_profile rc={rc}")
        try:
            yield
        finally:
            n = lib.axon_stop_nrt_profile(str(output_dir).encode())
            if n <= 0:
                print(f"WARNING: ntff capture wrote {n} files", file=sys.stderr)

    return _hook


def _install_prof_shim():
    if "antenv.axon_hooks" not in sys.modules:
        mod = types.ModuleType("antenv.axon_hooks")
        mod.set_axon_ntff_profile_hook = lambda h: _hook_holder.__setitem__(0, h)
        mod.get_axon_ntff_profile_hook = lambda: _hook_holder[0]
        sys.modules["antenv.axon_hooks"] = mod
    _hook_holder[0] = _ntff_profile_via_ctypes(_SO_PATH)
    import concourse.bass_utils as bu

    bu.upload_artifacts = lambda tmpdir: tmpdir


_install_prof_shim()

# ------------------------------------------------------------- wait-split pass
def _split_multi_waits(nc, maxw=1):
    """walrus in this container allows only one sync-wait per instruction;
    split extras onto nops inserted before the offender (same engine/block)."""

    def _remove_by_name(name):
        for f in nc.m.functions:
            for bb in f.blocks:
                for i, inst in enumerate(bb.instructions):
                    if inst.name == name:
                        lst = bb.instructions
                        del lst[i]
                        bb.instructions = lst
                        return inst
        raise KeyError(name)

    offenders = []
    for f in nc.m.functions:
        for bb in f.blocks:
            for inst in bb.instructions:
                si = inst.sync_info
                if si and si.on_wait and len(si.on_wait) > maxw:
                    offenders.append(inst.name)
    for name in offenders:
        target = None
        for f in nc.m.functions:
            for bb in f.blocks:
                for idx, inst in enumerate(bb.instructions):
                    if inst.name == name:
                        target = (bb, inst)
                        break
                if target:
                    break
            if target:
                break
        bb, inst = target
        waits = list(inst.sync_info.on_wait)
        updates = list(inst.sync_info.on_update or [])
        chunks = [waits[i:i + maxw] for i in range(0, len(waits), maxw)]
        nops = []
        for ch in chunks[:-1]:
            bnop = nc.engines[inst.engine].nop(nofuse=True, hint="waitsplit")
            nop_inst = _remove_by_name(bnop.ins.name)
            nop_inst.sync_info = mybir.SyncInfo(on_wait=ch, on_update=[])
            nops.append(nop_inst)
        inst.sync_info = mybir.SyncInfo(on_wait=chunks[-1], on_update=updates)
        lst = bb.instructions
        idx = next(i for i, x in enumerate(lst) if x.name == name)
        lst[idx:idx] = nops
        bb.instructions = lst
    return len(offenders)


# ------------------------------------------------------------------- constants
B, S, D = 1, 2048, 4096
H, G = 32, 8
HD = D // H            # 128
NC = 8                 # cores
HPC = H // NC          # q heads per core = 4
OC = D // NC           # out columns per core = 512
P = 128
KT = D // P            # 32 contraction tiles over D
SCH = 512              # sequence chunk width
NSC = S // SCH         # 4
NKB = S // P           # 16 key tiles of 128
NQS = SCH // P         # 4 q-subchunks per chunk
SCALE = float(1.0 / math.sqrt(float(HD)))

f32 = mybir.dt.float32
bf16 = mybir.dt.bfloat16

Copy = mybir.ActivationFunctionType.Copy
Exp = mybir.ActivationFunctionType.Exp


class _WoSched:
    """FIFO of deferred Wo work closures; emitted between attention ops."""

    def __init__(self):
        self.units = []

    def add(self, fn):
        self.units.append(fn)

    def take(self, n):
        k = min(n, len(self.units))
        for fn in self.units[:k]:
            fn()
        del self.units[:k]

    def drain(self):
        self.take(len(self.units))


def _build_program():
    nc = bass.Bass()
    xT = nc.declare_dram_parameter("xT", [P, NSC, KT, SCH], bf16, isOutput=False)
    wq = nc.declare_dram_parameter("wq", [P, HPC, KT, HD], bf16, isOutput=False)
    wk = nc.declare_dram_parameter("wk", [P, KT, HD], bf16, isOutput=False)
    wv = nc.declare_dram_parameter("wv", [P, KT, HD], bf16, isOutput=False)
    wo = nc.declare_dram_parameter("wo", [P, KT, OC], bf16, isOutput=False)
    cosT = nc.declare_dram_parameter("cosT", [HD, S], bf16, isOutput=False)
    sinT = nc.declare_dram_parameter("sinT", [HD, S], bf16, isOutput=False)
    tri = nc.declare_dram_parameter("tri", [P, P], bf16, isOutput=False)
    out = nc.declare_dram_parameter("out", [OC, S], bf16, isOutput=True)

    # one AllGather per (chunk, head-pair): in [2*P, SCH], out [NC*2*P, SCH]
    cc_ins = [[nc.dram_tensor(f"cc_in{sc}_{pr}", [2 * P, SCH], bf16)
               for pr in range(2)] for sc in range(NSC)]
    cc_outs = [[nc.dram_tensor(f"cc_out{sc}_{pr}", [NC * 2 * P, SCH], bf16,
                               addr_space="Shared")
                for pr in range(2)] for sc in range(NSC)]

    # global 128-row D-block kt = 4*rank + head -> (pair, tile within cc_out)
    def kt_loc(kt):
        r, h = kt // HPC, kt % HPC
        return h // 2, r * 2 + (h % 2)

    with tile.TileContext(nc) as tc:
        with (
            tc.tile_pool(name="singles", bufs=1) as singles,
            tc.tile_pool(name="xp", bufs=6) as xpp,
            tc.tile_pool(name="qts", bufs=6) as qtsp,
            tc.tile_pool(name="pt", bufs=20) as ptp,
            tc.tile_pool(name="work", bufs=4) as work,
            tc.tile_pool(name="stg", bufs=3) as stgp,
            tc.tile_pool(name="ccp", bufs=10) as ccp,
            tc.tile_pool(name="osb", bufs=3) as osbp,
            tc.tile_pool(name="ps", bufs=1, space="PSUM") as psp,
        ):
            # ---- constants loaded in compute-dependency order
            identf = singles.tile([P, P], f32)
            make_identity(nc, identf[:])
            ident = singles.tile([P, P], bf16)
            nc.vector.tensor_copy(ident[:], identf[:])
            trim = singles.tile([P, P], bf16)
            nc.sync.dma_start(out=trim[:], in_=tri[:])

            wk_sb = singles.tile([P, KT, HD], bf16)
            nc.sync.dma_start(out=wk_sb[:], in_=wk[:])
            wv_sb = singles.tile([P, KT, HD], bf16)
            nc.sync.dma_start(out=wv_sb[:], in_=wv[:])

            def issue_x(sc):
                tiles = []
                for g in range(KT // 8):
                    t = xpp.tile([P, 8, SCH], bf16, tag="xp", bufs=6)
                    nc.sync.dma_start(out=t[:], in_=xT[:, sc, 8 * g:8 * g + 8, :])
                    tiles.append(t)
                return tiles

            xtg_cur = issue_x(0)

            wq_sb = singles.tile([P, HPC, KT, HD], bf16)
            nc.sync.dma_start(out=wq_sb[:, 0], in_=wq[:, 0])
            cos_sb = singles.tile([HD, S], bf16)
            nc.sync.dma_start(out=cos_sb[:], in_=cosT[:])
            sin_sb = singles.tile([HD, S], bf16)
            nc.sync.dma_start(out=sin_sb[:], in_=sinT[:])
            for h in range(1, HPC):
                nc.sync.dma_start(out=wq_sb[:, h], in_=wq[:, h])
            wo_sb = singles.tile([P, KT, OC], bf16)
            nc.sync.dma_start(out=wo_sb[:], in_=wo[:])

            kT_all = singles.tile([HD, S], bf16)
            vp_all = singles.tile([P, NKB, HD + 1], bf16)
            nc.vector.memset(vp_all[:], 1.0)

            wos = _WoSched()

            # ---------- Wo split `spl`: two ob-pair passes over all 32 kt
            def queue_wo_split(spl):
                kts = sorted(range(KT), key=lambda kt: kt_loc(kt))
                for pss in (0, 1):
                    obs = (2 * pss, 2 * pss + 1)
                    accs = {}
                    ccts = {}
                    PF = 4

                    def issue_dma(i, kts=kts, ccts=ccts):
                        pr, t = kt_loc(kts[i])
                        cct = ccp.tile([P, SCH], bf16, tag="ccp")
                        nc.sync.dma_start(
                            out=cct[:],
                            in_=cc_outs[spl][pr][:].rearrange(
                                "(t p) s -> p t s", p=P)[:, t, :])
                        ccts[i] = cct

                    def emit(i, pss=pss, obs=obs, kts=kts, accs=accs,
                             ccts=ccts, PF=PF, issue_dma=issue_dma):
                        if i == 0:
                            for ob in obs:
                                accs[ob] = psp.tile([P, SCH], f32, tag="o",
                                                    bufs=2, name=f"wo{spl}_{ob}")
                            for j in range(min(PF, len(kts))):
                                issue_dma(j)
                        if i + PF < len(kts):
                            issue_dma(i + PF)
                        cct = ccts.pop(i)
                        first, last = i == 0, i == len(kts) - 1
                        for ob in obs:
                            nc.tensor.matmul(
                                accs[ob][:], wo_sb[:, kts[i], ob * P:(ob + 1) * P],
                                cct[:], start=first, stop=last)
                        if last:
                            for ob in obs:
                                ps_o = accs.pop(ob)
                                o_sb = osbp.tile([P, SCH], bf16, tag="osb")
                                nc.scalar.copy(o_sb[:], ps_o[:])
                                nc.scalar.dma_start(
                                    out=out[ob * P:(ob + 1) * P,
                                            spl * SCH:(spl + 1) * SCH],
                                    in_=o_sb[:])

                    for i in range(len(kts)):
                        wos.add(lambda i=i, emit=emit: emit(i))

            def rope_evict(ps_t, dst, tab0):
                """ps_t: PSUM [HD, SCH] pre-rope; writes dst [HD, SCH] bf16
                using rope tables at absolute position tab0."""
                rot = work.tile([HD, SCH], f32, tag="rot", bufs=2)
                nc.scalar.activation(out=rot[0:64, :], in_=ps_t[64:128, :],
                                     func=Copy, scale=-1.0)
                nc.scalar.activation(out=rot[64:128, :], in_=ps_t[0:64, :],
                                     func=Copy)
                m1 = work.tile([HD, SCH], f32, tag="m1", bufs=2)
                nc.vector.tensor_mul(m1[:], ps_t[:], cos_sb[:, tab0:tab0 + SCH])
                nc.vector.tensor_mul(rot[:], rot[:], sin_sb[:, tab0:tab0 + SCH])
                nc.vector.tensor_add(dst, m1[:], rot[:])

            # ================= chunk loop
            for sc in range(NSC):
                s0 = sc * SCH
                xtg = xtg_cur

                def xts(kt, xtg=xtg):
                    return xtg[kt // 8][:, kt % 8, :]

                # K projection -> RoPE -> kT_all
                ps_k = psp.tile([P, SCH], f32, tag="a", bufs=3)
                for kt in range(KT):
                    nc.tensor.matmul(ps_k[:], wk_sb[:, kt, :], xts(kt),
                                     start=(kt == 0), stop=(kt == KT - 1))
                rope_evict(ps_k, kT_all[:, s0:s0 + SCH], s0)

                # V projection -> transpose -> vp_all
                ps_v = psp.tile([P, SCH], f32, tag="a", bufs=3)
                for kt in range(KT):
                    nc.tensor.matmul(ps_v[:], wv_sb[:, kt, :], xts(kt),
                                     start=(kt == 0), stop=(kt == KT - 1))
                vc = work.tile([HD, SCH], bf16, tag="vc", bufs=2)
                nc.scalar.copy(vc[:], ps_v[:])

                # Q projections + RoPE (4 heads); V transposes after Q0
                qts = []
                for h in range(HPC):
                    ps_q = psp.tile([P, SCH], f32, tag="a", bufs=3)
                    for kt in range(KT):
                        nc.tensor.matmul(ps_q[:], wq_sb[:, h, kt, :], xts(kt),
                                         start=(kt == 0), stop=(kt == KT - 1))
                    qt = qtsp.tile([HD, SCH], bf16, tag="qts")
                    rope_evict(ps_q, qt[:], s0)
                    qts.append(qt)
                    if h == 0:
                        for j in range(NQS):
                            kb = sc * NQS + j
                            ps_vt = psp.tile([P, P], bf16, tag="t", bufs=2)
                            nc.tensor.transpose(
                                ps_vt[:], vc[:, j * P:(j + 1) * P], ident[:])
                            nc.vector.tensor_copy(vp_all[:, kb, 0:HD], ps_vt[:])

                # prefetch next chunk's x while attention runs
                if sc + 1 < NSC:
                    xtg_cur = issue_x(sc + 1)

                # attention; stage ctxT per head-pair, flush + AllGather
                nkb = NQS * sc + NQS  # key tiles visible to this chunk
                stage = None
                for h in range(HPC):
                    if h % 2 == 0:
                        stage = stgp.tile([P, 2, SCH], bf16, tag="stg")
                    # ---- scores + exp (+Wo filler every 4 tiles)
                    pts = []
                    for kb in range(nkb):
                        ps_s = psp.tile([P, SCH], f32, tag="a", bufs=3)
                        nc.tensor.matmul(ps_s[:], kT_all[:, kb * P:(kb + 1) * P],
                                         qts[h][:], start=True, stop=True)
                        pt = ptp.tile([P, SCH], bf16, tag="pt")
                        diag = kb - NQS * sc
                        off = diag * P if diag > 0 else 0
                        nc.scalar.activation(out=pt[:, off:], in_=ps_s[:, off:],
                                             func=Exp, scale=SCALE)
                        if 0 <= diag < NQS:
                            nc.vector.tensor_mul(
                                pt[:, diag * P:(diag + 1) * P],
                                pt[:, diag * P:(diag + 1) * P], trim[:])
                        pts.append(pt)
                        if kb % 4 == 3:
                            wos.take(1)

                    # ---- ctx per q-subchunk, normalize, transpose, stage
                    ctxns = []
                    for qh in range(NQS):
                        iqc = NQS * sc + qh
                        ps_c = psp.tile([P, SCH], f32, tag="a", bufs=3)
                        for kb in range(iqc + 1):
                            nc.tensor.matmul(
                                ps_c[:, 0:HD + 1],
                                pts[kb][:, qh * P:(qh + 1) * P],
                                vp_all[:, kb, :],
                                start=(kb == 0), stop=(kb == iqc))
                        rden = work.tile([P, 1], f32, tag="rden", bufs=4)
                        nc.vector.reciprocal(rden[:], ps_c[:, HD:HD + 1])
                        ctxn = work.tile([P, HD], bf16, tag="ctxn", bufs=4)
                        nc.vector.tensor_scalar_mul(ctxn[:], ps_c[:, 0:HD],
                                                    rden[:])
                        ctxns.append(ctxn)
                        wos.take(1)
                    for qh in range(NQS):
                        ps_t = psp.tile([P, P], bf16, tag="t", bufs=2)
                        nc.tensor.transpose(ps_t[:], ctxns[qh][:], ident[:])
                        nc.vector.tensor_copy(
                            stage[:, h % 2, qh * P:(qh + 1) * P], ps_t[:])
                    wos.take(2)

                    if h % 2 == 1:
                        pr = h // 2
                        nc.scalar.dma_start(
                            out=cc_ins[sc][pr][:].rearrange(
                                "(h p) c -> p h c", p=P),
                            in_=stage[:])
                        nc.gpsimd.collective_compute(
                            "AllGather",
                            mybir.AluOpType.bypass,
                            replica_groups=[list(range(NC))],
                            ins=[cc_ins[sc][pr][:]],
                            outs=[cc_outs[sc][pr][:]],
                        )
                        wos.take(2)

                # finish all Wo work whose AllGathers are long done, then
                # queue this chunk's split (consumable from next chunk)
                wos.drain()
                queue_wo_split(sc)

            # ================= tail: last split's Wo
            wos.drain()

    return nc


_PROGRAM_CACHE = {}


def _get_program():
    if "nc" not in _PROGRAM_CACHE:
        nc = _build_program()
        _split_multi_waits(nc, maxw=1)
        _PROGRAM_CACHE["nc"] = nc
    return _PROGRAM_CACHE["nc"]


def _rope_tables_T():
    inv_freq = (1.0 / (10000.0 ** (np.arange(0, HD, 2, dtype=np.float32) / HD))
                ).astype(np.float32)
    ang = np.arange(S, dtype=np.float32)[:, None] * inv_freq[None, :]
    ang = np.concatenate([ang, ang], axis=-1)  # [S, HD]
    cosT = np.ascontiguousarray(np.cos(ang).T).astype(ml_dtypes.bfloat16)
    sinT = np.ascontiguousarray(np.sin(ang).T).astype(ml_dtypes.bfloat16)
    return cosT, sinT


def _prep_in_maps(x, Wq, Wk, Wv, Wo):
    bf = ml_dtypes.bfloat16
    x2d = np.asarray(x, np.float32).reshape(S, D).astype(bf)
    # [D, S] -> [P, NSC, KT, SCH]
    xT_dev = np.ascontiguousarray(
        x2d.T.reshape(KT, P, NSC, SCH).transpose(1, 2, 0, 3))
    cosT, sinT = _rope_tables_T()
    tri_np = (np.arange(P)[:, None] <= np.arange(P)[None, :]).astype(bf)

    def wtiles(Wslice, width):
        return np.ascontiguousarray(
            np.asarray(Wslice, np.float32).astype(bf)
            .reshape(KT, P, width).transpose(1, 0, 2))

    in_maps = []
    for c in range(NC):
        wq_c = np.asarray(Wq[:, c * OC:(c + 1) * OC], np.float32).astype(bf)
        wq_t = np.ascontiguousarray(
            wq_c.reshape(KT, P, HPC, HD).transpose(1, 2, 0, 3))
        in_maps.append({
            "xT": xT_dev,
            "wq": wq_t,
            "wk": wtiles(Wk[:, c * HD:(c + 1) * HD], HD),
            "wv": wtiles(Wv[:, c * HD:(c + 1) * HD], HD),
            "wo": wtiles(Wo[:, c * OC:(c + 1) * OC], OC),
            "cosT": cosT,
            "sinT": sinT,
            "tri": tri_np,
        })
    return in_maps


def _run(inputs, trace=False):
    nc = _get_program()
    in_maps = _prep_in_maps(inputs["x"], inputs["Wq"], inputs["Wk"],
                            inputs["Wv"], inputs["Wo"])
    res = run_bass_kernel_spmd(nc, in_maps, core_ids=list(range(NC)),
                               trace=trace)
    out = np.concatenate(
        [np.asarray(res.results[c]["out"]).astype(np.float32).T
         for c in range(NC)], axis=1)
    return out.reshape(B, S, D).astype(np.float32), res


def kernel(**inputs):
    out, _ = _run(inputs, trace=False)
    return out


# revision 13
# speedup vs baseline: 1.2599x; 1.0242x over previous
"""GQA (B=1, S=2048, D=4096, H=32, G=8) on 8 TRN2 NeuronCores.

Tensor-parallel over heads: core c owns query heads 4c..4c+3 and KV group c.
v2: all-bf16 datapath (weights/x/k/q/probs/ctx/Wo and the AllGather wire),
SCH=512 sequence chunks, per-head-pair AllGathers issued mid-chunk, Wo
matmuls interleaved into the attention chunks as tensor-engine filler, and
output produced transposed ([OC, S]) so Wo keeps its weights stationary.
Host slices weights / transposes x / re-transposes the output.

Self-contained: no sibling imports; hardcoded shapes.
"""
import contextlib
import ctypes
import math
import os
import sys
import types

import ml_dtypes
import numpy as np

os.environ.setdefault("MYCRO_LOCAL_CACHE", "1")

for _p in ("/opt/trn_rl_repo", "/root/.axon_site/_ro/trn_rl_repo"):
    if _p not in sys.path and os.path.isdir(_p):
        sys.path.append(_p)

import concourse.bass as bass
import concourse.tile as tile
from concourse import mybir
from concourse.bass_utils import run_bass_kernel_spmd
from concourse.masks import make_identity

# ---------------------------------------------------------------- profiling shim
_SO_PATH = "/opt/axon/libaxon_pjrt.so"
_hook_holder = [None]


def _ntff_profile_via_ctypes(so_path):
    try:
        lib = ctypes.CDLL(so_path)
    except OSError:
        return None
    if not hasattr(lib, "axon_start_nrt_profile"):
        return None
    lib.axon_start_nrt_profile.argtypes = [
        ctypes.POINTER(ctypes.c_int64),
        ctypes.c_size_t,
    ]
    lib.axon_start_nrt_profile.restype = ctypes.c_int64
    lib.axon_stop_nrt_profile.argtypes = [ctypes.c_char_p]
    lib.axon_stop_nrt_profile.restype = ctypes.c_int64

    @contextlib.contextmanager
    def _hook(output_dir, device_ids):
        import jax

        jax.devices()
        if device_ids:
            ids = (ctypes.c_int64 * len(device_ids))(*device_ids)
            rc = lib.axon_start_nrt_profile(ids, len(device_ids))
        else:
            rc = lib.axon_start_nrt_profile(None, 0)
        if rc != 0:
            raise RuntimeError(f"axon_start_nrt_profile rc={rc}")
        try:
            yield
        finally:
            n = lib.axon_stop_nrt_profile(str(output_dir).encode())
            if n <= 0:
                print(f"WARNING: ntff capture wrote {n} files", file=sys.stderr)

    return _hook


def _install_prof_shim():
    if "antenv.axon_hooks" not in sys.modules:
        mod = types.ModuleType("antenv.axon_hooks")
        mod.set_axon_ntff_profile_hook = lambda h: _hook_holder.__setitem__(0, h)
        mod.get_axon_ntff_profile_hook = lambda: _hook_holder[0]
        sys.modules["antenv.axon_hooks"] = mod
    _hook_holder[0] = _ntff_profile_via_ctypes(_SO_PATH)
    import concourse.bass_utils as bu

    bu.upload_artifacts = lambda tmpdir: tmpdir


_install_prof_shim()

# ------------------------------------------------------------- wait-split pass
def _split_multi_waits(nc, maxw=1):
    """walrus in this container allows only one sync-wait per instruction;
    split extras onto nops inserted before the offender (same engine/block)."""

    def _remove_by_name(name):
        for f in nc.m.functions:
            for bb in f.blocks:
                for i, inst in enumerate(bb.instructions):
                    if inst.name == name:
                        lst = bb.instructions
                        del lst[i]
                        bb.instructions = lst
                        return inst
        raise KeyError(name)

    offenders = []
    for f in nc.m.functions:
        for bb in f.blocks:
            for inst in bb.instructions:
                si = inst.sync_info
                if si and si.on_wait and len(si.on_wait) > maxw:
                    offenders.append(inst.name)
    for name in offenders:
        target = None
        for f in nc.m.functions:
            for bb in f.blocks:
                for idx, inst in enumerate(bb.instructions):
                    if inst.name == name:
                        target = (bb, inst)
                        break
                if target:
                    break
            if target:
                break
        bb, inst = target
        waits = list(inst.sync_info.on_wait)
        updates = list(inst.sync_info.on_update or [])
        chunks = [waits[i:i + maxw] for i in range(0, len(waits), maxw)]
        nops = []
        for ch in chunks[:-1]:
            bnop = nc.engines[inst.engine].nop(nofuse=True, hint="waitsplit")
            nop_inst = _remove_by_name(bnop.ins.name)
            nop_inst.sync_info = mybir.SyncInfo(on_wait=ch, on_update=[])
            nops.append(nop_inst)
        inst.sync_info = mybir.SyncInfo(on_wait=chunks[-1], on_update=updates)
        lst = bb.instructions
        idx = next(i for i, x in enumerate(lst) if x.name == name)
        lst[idx:idx] = nops
        bb.instructions = lst
    return len(offenders)


# ------------------------------------------------------------------- constants
B, S, D = 1, 2048, 4096
H, G = 32, 8
HD = D // H            # 128
NC = 8                 # cores
HPC = H // NC          # q heads per core = 4
OC = D // NC           # out columns per core = 512
P = 128
KT = D // P            # 32 contraction tiles over D
SCH = 512              # sequence chunk width
NSC = S // SCH         # 4
NKB = S // P           # 16 key tiles of 128
NQS = SCH // P         # 4 q-subchunks per chunk
SCALE = float(1.0 / math.sqrt(float(HD)))

f32 = mybir.dt.float32
bf16 = mybir.dt.bfloat16

Copy = mybir.ActivationFunctionType.Copy
Exp = mybir.ActivationFunctionType.Exp


class _WoSched:
    """FIFO of deferred Wo work closures; emitted between attention ops."""

    def __init__(self):
        self.units = []

    def add(self, fn):
        self.units.append(fn)

    def take(self, n):
        k = min(n, len(self.units))
        for fn in self.units[:k]:
            fn()
        del self.units[:k]

    def drain(self):
        self.take(len(self.units))


def _build_program():
    nc = bass.Bass()
    xT = nc.declare_dram_parameter("xT", [P, NSC, KT, SCH], bf16, isOutput=False)
    wq = nc.declare_dram_parameter("wq", [P, HPC, KT, HD], bf16, isOutput=False)
    wk = nc.declare_dram_parameter("wk", [P, KT, HD], bf16, isOutput=False)
    wv = nc.declare_dram_parameter("wv", [P, KT, HD], bf16, isOutput=False)
    wo = nc.declare_dram_parameter("wo", [P, KT, OC], bf16, isOutput=False)
    cosT = nc.declare_dram_parameter("cosT", [HD, S], bf16, isOutput=False)
    sinT = nc.declare_dram_parameter("sinT", [HD, S], bf16, isOutput=False)
    tri = nc.declare_dram_parameter("tri", [P, P], bf16, isOutput=False)
    out = nc.declare_dram_parameter("out", [OC, S], bf16, isOutput=True)

    # one AllGather per (chunk, head-pair): in [2*P, SCH], out [NC*2*P, SCH]
    cc_ins = [[nc.dram_tensor(f"cc_in{sc}_{pr}", [2 * P, SCH], bf16)
               for pr in range(2)] for sc in range(NSC)]
    cc_outs = [[nc.dram_tensor(f"cc_out{sc}_{pr}", [NC * 2 * P, SCH], bf16,
                               addr_space="Shared")
                for pr in range(2)] for sc in range(NSC)]

    # global 128-row D-block kt = 4*rank + head -> (pair, tile within cc_out)
    def kt_loc(kt):
        r, h = kt // HPC, kt % HPC
        return h // 2, r * 2 + (h % 2)

    with tile.TileContext(nc) as tc:
        with (
            tc.tile_pool(name="singles", bufs=1) as singles,
            tc.tile_pool(name="xp", bufs=6) as xpp,
            tc.tile_pool(name="qts", bufs=6) as qtsp,
            tc.tile_pool(name="pt", bufs=20) as ptp,
            tc.tile_pool(name="work", bufs=4) as work,
            tc.tile_pool(name="stg", bufs=3) as stgp,
            tc.tile_pool(name="ccp", bufs=12) as ccp,
            tc.tile_pool(name="osb", bufs=3) as osbp,
            tc.tile_pool(name="ps", bufs=1, space="PSUM") as psp,
        ):
            # ---- constants loaded in compute-dependency order
            identf = singles.tile([P, P], f32)
            make_identity(nc, identf[:])
            ident = singles.tile([P, P], bf16)
            nc.vector.tensor_copy(ident[:], identf[:])
            trim = singles.tile([P, P], bf16)
            nc.sync.dma_start(out=trim[:], in_=tri[:])

            def issue_x(sc):
                tiles = []
                for g in range(KT // 8):
                    t = xpp.tile([P, 8, SCH], bf16, tag="xp", bufs=6)
                    nc.sync.dma_start(out=t[:], in_=xT[:, sc, 8 * g:8 * g + 8, :])
                    tiles.append(t)
                return tiles

            # stagger wk pieces with x pieces so the first projections can
            # begin as soon as ~0.5 MB has landed
            wk_sb = singles.tile([P, KT, HD], bf16)
            wv_sb = singles.tile([P, KT, HD], bf16)
            xtg_cur = []
            for g in range(KT // 8):
                nc.sync.dma_start(out=wk_sb[:, 8 * g:8 * g + 8, :],
                                  in_=wk[:, 8 * g:8 * g + 8, :])
                t = xpp.tile([P, 8, SCH], bf16, tag="xp", bufs=6)
                nc.sync.dma_start(out=t[:], in_=xT[:, 0, 8 * g:8 * g + 8, :])
                xtg_cur.append(t)
            nc.sync.dma_start(out=wv_sb[:], in_=wv[:])

            wq_sb = singles.tile([P, HPC, KT, HD], bf16)
            nc.sync.dma_start(out=wq_sb[:, 0], in_=wq[:, 0])
            cos_sb = singles.tile([HD, S], bf16)
            nc.sync.dma_start(out=cos_sb[:], in_=cosT[:])
            sin_sb = singles.tile([HD, S], bf16)
            nc.sync.dma_start(out=sin_sb[:], in_=sinT[:])
            for h in range(1, HPC):
                nc.sync.dma_start(out=wq_sb[:, h], in_=wq[:, h])
            wo_sb = singles.tile([P, KT, OC], bf16)
            nc.sync.dma_start(out=wo_sb[:], in_=wo[:])

            kT_all = singles.tile([HD, S], bf16)
            vp_all = singles.tile([P, NKB, HD + 1], bf16)
            nc.vector.memset(vp_all[:], 1.0)

            wos = _WoSched()

            # ---------- Wo split `spl`: two ob-pair passes over all 32 kt
            def queue_wo_split(spl):
                kts = sorted(range(KT), key=lambda kt: kt_loc(kt))
                for pss in (0, 1):
                    obs = (2 * pss, 2 * pss + 1)
                    accs = {}
                    ccts = {}
                    PF = 6

                    def issue_dma(i, kts=kts, ccts=ccts):
                        pr, t = kt_loc(kts[i])
                        cct = ccp.tile([P, SCH], bf16, tag="ccp")
                        nc.sync.dma_start(
                            out=cct[:],
                            in_=cc_outs[spl][pr][:].rearrange(
                                "(t p) s -> p t s", p=P)[:, t, :])
                        ccts[i] = cct

                    def emit(i, pss=pss, obs=obs, kts=kts, accs=accs,
                             ccts=ccts, PF=PF, issue_dma=issue_dma):
                        if i == 0:
                            for ob in obs:
                                accs[ob] = psp.tile([P, SCH], f32, tag="o",
                                                    bufs=2, name=f"wo{spl}_{ob}")
                            for j in range(min(PF, len(kts))):
                                issue_dma(j)
                        if i + PF < len(kts):
                            issue_dma(i + PF)
                        cct = ccts.pop(i)
                        first, last = i == 0, i == len(kts) - 1
                        for ob in obs:
                            nc.tensor.matmul(
                                accs[ob][:], wo_sb[:, kts[i], ob * P:(ob + 1) * P],
                                cct[:], start=first, stop=last)
                        if last:
                            for ob in obs:
                                ps_o = accs.pop(ob)
                                o_sb = osbp.tile([P, SCH], bf16, tag="osb")
                                nc.vector.tensor_copy(o_sb[:], ps_o[:])
                                nc.scalar.dma_start(
                                    out=out[ob * P:(ob + 1) * P,
                                            spl * SCH:(spl + 1) * SCH],
                                    in_=o_sb[:])

                    for i in range(len(kts)):
                        wos.add(lambda i=i, emit=emit: emit(i))

            def rope_evict(ps_t, dst, tab0):
                """ps_t: PSUM [HD, SCH] pre-rope; writes dst [HD, SCH] bf16
                using rope tables at absolute position tab0."""
                rot = work.tile([HD, SCH], f32, tag="rot", bufs=2)
                nc.vector.tensor_scalar_mul(rot[0:64, :], ps_t[64:128, :], -1.0)
                nc.vector.tensor_copy(rot[64:128, :], ps_t[0:64, :])
                m1 = work.tile([HD, SCH], f32, tag="m1", bufs=2)
                nc.vector.tensor_mul(m1[:], ps_t[:], cos_sb[:, tab0:tab0 + SCH])
                nc.vector.tensor_mul(rot[:], rot[:], sin_sb[:, tab0:tab0 + SCH])
                nc.vector.tensor_add(dst, m1[:], rot[:])

            # ================= chunk loop
            for sc in range(NSC):
                s0 = sc * SCH
                xtg = xtg_cur

                def xts(kt, xtg=xtg):
                    return xtg[kt // 8][:, kt % 8, :]

                # K projection -> RoPE -> kT_all
                ps_k = psp.tile([P, SCH], f32, tag="a", bufs=3)
                for kt in range(KT):
                    nc.tensor.matmul(ps_k[:], wk_sb[:, kt, :], xts(kt),
                                     start=(kt == 0), stop=(kt == KT - 1))
                rope_evict(ps_k, kT_all[:, s0:s0 + SCH], s0)

                # V projection -> transpose -> vp_all
                ps_v = psp.tile([P, SCH], f32, tag="a", bufs=3)
                for kt in range(KT):
                    nc.tensor.matmul(ps_v[:], wv_sb[:, kt, :], xts(kt),
                                     start=(kt == 0), stop=(kt == KT - 1))
                vc = work.tile([HD, SCH], bf16, tag="vc", bufs=2)
                nc.vector.tensor_copy(vc[:], ps_v[:])

                # Q projections + RoPE (4 heads); V transposes after Q0
                qts = []
                for h in range(HPC):
                    ps_q = psp.tile([P, SCH], f32, tag="a", bufs=3)
                    for kt in range(KT):
                        nc.tensor.matmul(ps_q[:], wq_sb[:, h, kt, :], xts(kt),
                                         start=(kt == 0), stop=(kt == KT - 1))
                    qt = qtsp.tile([HD, SCH], bf16, tag="qts")
                    rope_evict(ps_q, qt[:], s0)
                    qts.append(qt)
                    if h == 0:
                        for j in range(NQS):
                            kb = sc * NQS + j
                            ps_vt = psp.tile([P, P], bf16, tag="t", bufs=2)
                            nc.tensor.transpose(
                                ps_vt[:], vc[:, j * P:(j + 1) * P], ident[:])
                            nc.vector.tensor_copy(vp_all[:, kb, 0:HD], ps_vt[:])

                # prefetch next chunk's x while attention runs
                if sc + 1 < NSC:
                    xtg_cur = issue_x(sc + 1)

                # attention; stage ctxT per head-pair, flush + AllGather.
                # ctx groups are interleaved right behind their diagonal
                # score tile, with Wo filler pacing the scalar-engine exps.
                nkb = NQS * sc + NQS  # key tiles visible to this chunk
                stage = None
                for h in range(HPC):
                    if h % 2 == 0:
                        stage = stgp.tile([P, 2, SCH], bf16, tag="stg")
                    pts = []
                    ctxns = []

                    def ctx_group(qh, pts=pts, ctxns=ctxns):
                        iqc = NQS * sc + qh
                        ps_c = psp.tile([P, SCH], f32, tag="a", bufs=3,
                                        name=f"psc{sc}_{qh}")
                        for kb in range(iqc + 1):
                            nc.tensor.matmul(
                                ps_c[:, 0:HD + 1],
                                pts[kb][:, qh * P:(qh + 1) * P],
                                vp_all[:, kb, :],
                                start=(kb == 0), stop=(kb == iqc))
                        rden = work.tile([P, 1], f32, tag="rden", bufs=4)
                        nc.vector.reciprocal(rden[:], ps_c[:, HD:HD + 1])
                        ctxn = work.tile([P, HD], bf16, tag="ctxn", bufs=4)
                        nc.vector.tensor_scalar_mul(ctxn[:], ps_c[:, 0:HD],
                                                    rden[:])
                        ctxns.append(ctxn)

                    for kb in range(nkb):
                        ps_s = psp.tile([P, SCH], f32, tag="a", bufs=3)
                        nc.tensor.matmul(ps_s[:], kT_all[:, kb * P:(kb + 1) * P],
                                         qts[h][:], start=True, stop=True)
                        pt = ptp.tile([P, SCH], bf16, tag="pt")
                        diag = kb - NQS * sc
                        off = diag * P if diag > 0 else 0
                        nc.scalar.activation(out=pt[:, off:], in_=ps_s[:, off:],
                                             func=Exp, scale=SCALE)
                        if 0 <= diag < NQS:
                            nc.vector.tensor_mul(
                                pt[:, diag * P:(diag + 1) * P],
                                pt[:, diag * P:(diag + 1) * P], trim[:])
                        pts.append(pt)
                        if kb % 2 == 1:
                            wos.take(1)
                        if diag >= 0:
                            wos.take(1)
                            ctx_group(diag)

                    for qh in range(NQS):
                        ps_t = psp.tile([P, P], bf16, tag="t", bufs=2)
                        nc.tensor.transpose(ps_t[:], ctxns[qh][:], ident[:])
                        nc.vector.tensor_copy(
                            stage[:, h % 2, qh * P:(qh + 1) * P], ps_t[:])
                    wos.take(2)

                    if h % 2 == 1:
                        pr = h // 2
                        nc.scalar.dma_start(
                            out=cc_ins[sc][pr][:].rearrange(
                                "(h p) c -> p h c", p=P),
                            in_=stage[:])
                        nc.gpsimd.collective_compute(
                            "AllGather",
                            mybir.AluOpType.bypass,
                            replica_groups=[list(range(NC))],
                            ins=[cc_ins[sc][pr][:]],
                            outs=[cc_outs[sc][pr][:]],
                        )
                        wos.take(2)

                # finish all Wo work whose AllGathers are long done, then
                # queue this chunk's split (consumable from next chunk)
                wos.drain()
                queue_wo_split(sc)

            # ================= tail: last split's Wo
            wos.drain()

    return nc


_PROGRAM_CACHE = {}


def _get_program():
    if "nc" not in _PROGRAM_CACHE:
        nc = _build_program()
        _split_multi_waits(nc, maxw=1)
        _PROGRAM_CACHE["nc"] = nc
    return _PROGRAM_CACHE["nc"]


def _rope_tables_T():
    inv_freq = (1.0 / (10000.0 ** (np.arange(0, HD, 2, dtype=np.float32) / HD))
                ).astype(np.float32)
    ang = np.arange(S, dtype=np.float32)[:, None] * inv_freq[None, :]
    ang = np.concatenate([ang, ang], axis=-1)  # [S, HD]
    cosT = np.ascontiguousarray(np.cos(ang).T).astype(ml_dtypes.bfloat16)
    sinT = np.ascontiguousarray(np.sin(ang).T).astype(ml_dtypes.bfloat16)
    return cosT, sinT


def _prep_in_maps(x, Wq, Wk, Wv, Wo):
    bf = ml_dtypes.bfloat16
    x2d = np.asarray(x, np.float32).reshape(S, D).astype(bf)
    # [D, S] -> [P, NSC, KT, SCH]
    xT_dev = np.ascontiguousarray(
        x2d.T.reshape(KT, P, NSC, SCH).transpose(1, 2, 0, 3))
    cosT, sinT = _rope_tables_T()
    tri_np = (np.arange(P)[:, None] <= np.arange(P)[None, :]).astype(bf)

    def wtiles(Wslice, width):
        return np.ascontiguousarray(
            np.asarray(Wslice, np.float32).astype(bf)
            .reshape(KT, P, width).transpose(1, 0, 2))

    in_maps = []
    for c in range(NC):
        wq_c = np.asarray(Wq[:, c * OC:(c + 1) * OC], np.float32).astype(bf)
        wq_t = np.ascontiguousarray(
            wq_c.reshape(KT, P, HPC, HD).transpose(1, 2, 0, 3))
        in_maps.append({
            "xT": xT_dev,
            "wq": wq_t,
            "wk": wtiles(Wk[:, c * HD:(c + 1) * HD], HD),
            "wv": wtiles(Wv[:, c * HD:(c + 1) * HD], HD),
            "wo": wtiles(Wo[:, c * OC:(c + 1) * OC], OC),
            "cosT": cosT,
            "sinT": sinT,
            "tri": tri_np,
        })
    return in_maps


def _run(inputs, trace=False):
    nc = _get_program()
    in_maps = _prep_in_maps(inputs["x"], inputs["Wq"], inputs["Wk"],
                            inputs["Wv"], inputs["Wo"])
    res = run_bass_kernel_spmd(nc, in_maps, core_ids=list(range(NC)),
                               trace=trace)
    out = np.concatenate(
        [np.asarray(res.results[c]["out"]).astype(np.float32).T
         for c in range(NC)], axis=1)
    return out.reshape(B, S, D).astype(np.float32), res


def kernel(**inputs):
    out, _ = _run(inputs, trace=False)
    return out


# revision 18
# speedup vs baseline: 1.3126x; 1.0418x over previous
"""GQA (B=1, S=2048, D=4096, H=32, G=8) on 8 TRN2 NeuronCores.

Tensor-parallel over heads: core c owns query heads 4c..4c+3 and KV group c.
v2: all-bf16 datapath (weights/x/k/q/probs/ctx/Wo and the AllGather wire),
SCH=512 sequence chunks, per-head-pair AllGathers issued mid-chunk, Wo
matmuls interleaved into the attention chunks as tensor-engine filler, and
output produced transposed ([OC, S]) so Wo keeps its weights stationary.
Host slices weights / transposes x / re-transposes the output.

Self-contained: no sibling imports; hardcoded shapes.
"""
import contextlib
import ctypes
import math
import os
import sys
import types

import ml_dtypes
import numpy as np

os.environ.setdefault("MYCRO_LOCAL_CACHE", "1")

for _p in ("/opt/trn_rl_repo", "/root/.axon_site/_ro/trn_rl_repo"):
    if _p not in sys.path and os.path.isdir(_p):
        sys.path.append(_p)

import concourse.bass as bass
import concourse.tile as tile
from concourse import mybir
from concourse.bass_utils import run_bass_kernel_spmd
from concourse.masks import make_identity

# ---------------------------------------------------------------- profiling shim
_SO_PATH = "/opt/axon/libaxon_pjrt.so"
_hook_holder = [None]


def _ntff_profile_via_ctypes(so_path):
    try:
        lib = ctypes.CDLL(so_path)
    except OSError:
        return None
    if not hasattr(lib, "axon_start_nrt_profile"):
        return None
    lib.axon_start_nrt_profile.argtypes = [
        ctypes.POINTER(ctypes.c_int64),
        ctypes.c_size_t,
    ]
    lib.axon_start_nrt_profile.restype = ctypes.c_int64
    lib.axon_stop_nrt_profile.argtypes = [ctypes.c_char_p]
    lib.axon_stop_nrt_profile.restype = ctypes.c_int64

    @contextlib.contextmanager
    def _hook(output_dir, device_ids):
        import jax

        jax.devices()
        if device_ids:
            ids = (ctypes.c_int64 * len(device_ids))(*device_ids)
            rc = lib.axon_start_nrt_profile(ids, len(device_ids))
        else:
            rc = lib.axon_start_nrt_profile(None, 0)
        if rc != 0:
            raise RuntimeError(f"axon_start_nrt_profile rc={rc}")
        try:
            yield
        finally:
            n = lib.axon_stop_nrt_profile(str(output_dir).encode())
            if n <= 0:
                print(f"WARNING: ntff capture wrote {n} files", file=sys.stderr)

    return _hook


def _install_prof_shim():
    if "antenv.axon_hooks" not in sys.modules:
        mod = types.ModuleType("antenv.axon_hooks")
        mod.set_axon_ntff_profile_hook = lambda h: _hook_holder.__setitem__(0, h)
        mod.get_axon_ntff_profile_hook = lambda: _hook_holder[0]
        sys.modules["antenv.axon_hooks"] = mod
    _hook_holder[0] = _ntff_profile_via_ctypes(_SO_PATH)
    import concourse.bass_utils as bu

    bu.upload_artifacts = lambda tmpdir: tmpdir


_install_prof_shim()

# ------------------------------------------------------------- wait-split pass
def _split_multi_waits(nc, maxw=1):
    """walrus in this container allows only one sync-wait per instruction;
    split extras onto nops inserted before the offender (same engine/block)."""

    def _remove_by_name(name):
        for f in nc.m.functions:
            for bb in f.blocks:
                for i, inst in enumerate(bb.instructions):
                    if inst.name == name:
                        lst = bb.instructions
                        del lst[i]
                        bb.instructions = lst
                        return inst
        raise KeyError(name)

    offenders = []
    for f in nc.m.functions:
        for bb in f.blocks:
            for inst in bb.instructions:
                si = inst.sync_info
                if si and si.on_wait and len(si.on_wait) > maxw:
                    offenders.append(inst.name)
    for name in offenders:
        target = None
        for f in nc.m.functions:
            for bb in f.blocks:
                for idx, inst in enumerate(bb.instructions):
                    if inst.name == name:
                        target = (bb, inst)
                        break
                if target:
                    break
            if target:
                break
        bb, inst = target
        waits = list(inst.sync_info.on_wait)
        updates = list(inst.sync_info.on_update or [])
        chunks = [waits[i:i + maxw] for i in range(0, len(waits), maxw)]
        nops = []
        for ch in chunks[:-1]:
            bnop = nc.engines[inst.engine].nop(nofuse=True, hint="waitsplit")
            nop_inst = _remove_by_name(bnop.ins.name)
            nop_inst.sync_info = mybir.SyncInfo(on_wait=ch, on_update=[])
            nops.append(nop_inst)
        inst.sync_info = mybir.SyncInfo(on_wait=chunks[-1], on_update=updates)
        lst = bb.instructions
        idx = next(i for i, x in enumerate(lst) if x.name == name)
        lst[idx:idx] = nops
        bb.instructions = lst
    return len(offenders)


# ------------------------------------------------------------------- constants
B, S, D = 1, 2048, 4096
H, G = 32, 8
HD = D // H            # 128
NC = 8                 # cores
HPC = H // NC          # q heads per core = 4
OC = D // NC           # out columns per core = 512
P = 128
KT = D // P            # 32 contraction tiles over D
SCH = 512              # sequence chunk width
NSC = S // SCH         # 4
NKB = S // P           # 16 key tiles of 128
NQS = SCH // P         # 4 q-subchunks per chunk
SCALE = float(1.0 / math.sqrt(float(HD)))

f32 = mybir.dt.float32
bf16 = mybir.dt.bfloat16

Copy = mybir.ActivationFunctionType.Copy
Exp = mybir.ActivationFunctionType.Exp


class _WoSched:
    """FIFO of deferred Wo work closures; emitted between attention ops."""

    def __init__(self):
        self.units = []

    def add(self, fn):
        self.units.append(fn)

    def take(self, n):
        k = min(n, len(self.units))
        for fn in self.units[:k]:
            fn()
        del self.units[:k]

    def drain(self):
        self.take(len(self.units))


def _build_program():
    nc = bass.Bass()
    xT = nc.declare_dram_parameter("xT", [P, NSC, KT, SCH], bf16, isOutput=False)
    wq = nc.declare_dram_parameter("wq", [P, HPC, KT, HD], bf16, isOutput=False)
    wk = nc.declare_dram_parameter("wk", [P, KT, HD], bf16, isOutput=False)
    wv = nc.declare_dram_parameter("wv", [P, KT, HD], bf16, isOutput=False)
    wo = nc.declare_dram_parameter("wo", [P, KT, OC], bf16, isOutput=False)
    cosT = nc.declare_dram_parameter("cosT", [HD, S], bf16, isOutput=False)
    sinT = nc.declare_dram_parameter("sinT", [HD, S], bf16, isOutput=False)
    tri = nc.declare_dram_parameter("tri", [P, P], bf16, isOutput=False)
    out = nc.declare_dram_parameter("out", [OC, S], bf16, isOutput=True)

    # one AllGather per (chunk, head-pair): in [2*P, SCH], out [NC*2*P, SCH];
    # the last chunk flushes per-head (4 smaller AGs) to shrink the tail
    cc_ins = [[nc.dram_tensor(f"cc_in{sc}_{pr}", [2 * P, SCH], bf16)
               for pr in range(2)] for sc in range(NSC - 1)]
    cc_outs = [[nc.dram_tensor(f"cc_out{sc}_{pr}", [NC * 2 * P, SCH], bf16,
                               addr_space="Shared")
                for pr in range(2)] for sc in range(NSC - 1)]
    cc3_ins = [nc.dram_tensor(f"cc3_in{h}", [P, SCH], bf16)
               for h in range(HPC)]
    cc3_outs = [nc.dram_tensor(f"cc3_out{h}", [NC * P, SCH], bf16,
                               addr_space="Shared") for h in range(HPC)]

    # global 128-row D-block kt = 4*rank + head -> (pair, tile within cc_out)
    def kt_loc(kt):
        r, h = kt // HPC, kt % HPC
        return h // 2, r * 2 + (h % 2)

    with tile.TileContext(nc) as tc:
        with (
            tc.tile_pool(name="singles", bufs=1) as singles,
            tc.tile_pool(name="xp", bufs=6) as xpp,
            tc.tile_pool(name="qts", bufs=6) as qtsp,
            tc.tile_pool(name="pt", bufs=20) as ptp,
            tc.tile_pool(name="work", bufs=4) as work,
            tc.tile_pool(name="stg", bufs=3) as stgp,
            tc.tile_pool(name="ccp", bufs=12) as ccp,
            tc.tile_pool(name="osb", bufs=3) as osbp,
            tc.tile_pool(name="ps", bufs=1, space="PSUM") as psp,
        ):
            # ---- constants loaded in compute-dependency order
            identf = singles.tile([P, P], f32)
            make_identity(nc, identf[:])
            ident = singles.tile([P, P], bf16)
            nc.vector.tensor_copy(ident[:], identf[:])
            trim = singles.tile([P, P], bf16)
            nc.sync.dma_start(out=trim[:], in_=tri[:])

            def issue_x(sc):
                tiles = []
                for g in range(KT // 8):
                    t = xpp.tile([P, 8, SCH], bf16, tag="xp", bufs=6)
                    nc.sync.dma_start(out=t[:], in_=xT[:, sc, 8 * g:8 * g + 8, :])
                    tiles.append(t)
                return tiles

            # stagger wk pieces with x pieces so the first projections can
            # begin as soon as ~0.5 MB has landed
            wk_sb = singles.tile([P, KT, HD], bf16)
            wv_sb = singles.tile([P, KT, HD], bf16)
            xtg_cur = []
            for g in range(KT // 8):
                nc.sync.dma_start(out=wk_sb[:, 8 * g:8 * g + 8, :],
                                  in_=wk[:, 8 * g:8 * g + 8, :])
                t = xpp.tile([P, 8, SCH], bf16, tag="xp", bufs=6)
                nc.sync.dma_start(out=t[:], in_=xT[:, 0, 8 * g:8 * g + 8, :])
                xtg_cur.append(t)
            nc.sync.dma_start(out=wv_sb[:], in_=wv[:])

            wq_sb = singles.tile([P, HPC, KT, HD], bf16)
            nc.sync.dma_start(out=wq_sb[:, 0], in_=wq[:, 0])
            cos_sb = singles.tile([HD, S], bf16)
            nc.sync.dma_start(out=cos_sb[:], in_=cosT[:])
            sin_sb = singles.tile([HD, S], bf16)
            nc.sync.dma_start(out=sin_sb[:], in_=sinT[:])
            for h in range(1, HPC):
                nc.sync.dma_start(out=wq_sb[:, h], in_=wq[:, h])
            wo_sb = singles.tile([P, KT, OC], bf16)
            nc.sync.dma_start(out=wo_sb[:], in_=wo[:])

            kT_all = singles.tile([HD, S], bf16)
            vp_all = singles.tile([P, NKB, HD + 1], bf16)
            nc.vector.memset(vp_all[:], 1.0)

            wos = _WoSched()

            # ---------- Wo split `spl`: two ob-pair passes over all 32 kt
            def queue_wo_split(spl):
                kts = sorted(range(KT), key=lambda kt: kt_loc(kt))
                for pss in (0, 1):
                    obs = (2 * pss, 2 * pss + 1)
                    accs = {}
                    ccts = {}
                    PF = 6

                    def issue_dma(i, kts=kts, ccts=ccts):
                        pr, t = kt_loc(kts[i])
                        cct = ccp.tile([P, SCH], bf16, tag="ccp")
                        nc.sync.dma_start(
                            out=cct[:],
                            in_=cc_outs[spl][pr][:].rearrange(
                                "(t p) s -> p t s", p=P)[:, t, :])
                        ccts[i] = cct

                    def emit(i, pss=pss, obs=obs, kts=kts, accs=accs,
                             ccts=ccts, PF=PF, issue_dma=issue_dma):
                        if i == 0:
                            for ob in obs:
                                accs[ob] = psp.tile([P, SCH], f32, tag="o",
                                                    bufs=2, name=f"wo{spl}_{ob}")
                            for j in range(min(PF, len(kts))):
                                issue_dma(j)
                        if i + PF < len(kts):
                            issue_dma(i + PF)
                        cct = ccts.pop(i)
                        first, last = i == 0, i == len(kts) - 1
                        for ob in obs:
                            nc.tensor.matmul(
                                accs[ob][:], wo_sb[:, kts[i], ob * P:(ob + 1) * P],
                                cct[:], start=first, stop=last)
                        if last:
                            for ob in obs:
                                ps_o = accs.pop(ob)
                                o_sb = osbp.tile([P, SCH], bf16, tag="osb")
                                nc.vector.tensor_copy(o_sb[:], ps_o[:])
                                nc.scalar.dma_start(
                                    out=out[ob * P:(ob + 1) * P,
                                            spl * SCH:(spl + 1) * SCH],
                                    in_=o_sb[:])

                    for i in range(len(kts)):
                        wos.add(lambda i=i, emit=emit: emit(i))

            def rope_evict(ps_t, dst, tab0):
                """ps_t: PSUM [HD, SCH] pre-rope; writes dst [HD, SCH] bf16
                using rope tables at absolute position tab0."""
                rot = work.tile([HD, SCH], f32, tag="rot", bufs=2)
                nc.vector.tensor_scalar_mul(rot[0:64, :], ps_t[64:128, :], -1.0)
                nc.vector.tensor_copy(rot[64:128, :], ps_t[0:64, :])
                m1 = work.tile([HD, SCH], f32, tag="m1", bufs=2)
                nc.vector.tensor_mul(m1[:], ps_t[:], cos_sb[:, tab0:tab0 + SCH])
                nc.vector.tensor_mul(rot[:], rot[:], sin_sb[:, tab0:tab0 + SCH])
                nc.vector.tensor_add(dst, m1[:], rot[:])

            # ================= chunk loop
            for sc in range(NSC):
                s0 = sc * SCH
                xtg = xtg_cur

                def xts(kt, xtg=xtg):
                    return xtg[kt // 8][:, kt % 8, :]

                # K projection -> RoPE -> kT_all
                ps_k = psp.tile([P, SCH], f32, tag="a", bufs=3)
                for kt in range(KT):
                    nc.tensor.matmul(ps_k[:], wk_sb[:, kt, :], xts(kt),
                                     start=(kt == 0), stop=(kt == KT - 1))
                rope_evict(ps_k, kT_all[:, s0:s0 + SCH], s0)

                # V projection -> transpose -> vp_all
                ps_v = psp.tile([P, SCH], f32, tag="a", bufs=3)
                for kt in range(KT):
                    nc.tensor.matmul(ps_v[:], wv_sb[:, kt, :], xts(kt),
                                     start=(kt == 0), stop=(kt == KT - 1))
                vc = work.tile([HD, SCH], bf16, tag="vc", bufs=2)
                nc.vector.tensor_copy(vc[:], ps_v[:])

                # Q projections + RoPE (4 heads); V transposes after Q0
                qts = []
                for h in range(HPC):
                    ps_q = psp.tile([P, SCH], f32, tag="a", bufs=3)
                    for kt in range(KT):
                        nc.tensor.matmul(ps_q[:], wq_sb[:, h, kt, :], xts(kt),
                                         start=(kt == 0), stop=(kt == KT - 1))
                    qt = qtsp.tile([HD, SCH], bf16, tag="qts")
                    rope_evict(ps_q, qt[:], s0)
                    qts.append(qt)
                    if h == 0:
                        for j in range(NQS):
                            kb = sc * NQS + j
                            ps_vt = psp.tile([P, P], bf16, tag="t", bufs=2)
                            nc.tensor.transpose(
                                ps_vt[:], vc[:, j * P:(j + 1) * P], ident[:])
                            nc.vector.tensor_copy(vp_all[:, kb, 0:HD], ps_vt[:])

                # prefetch next chunk's x while attention runs
                if sc + 1 < NSC:
                    xtg_cur = issue_x(sc + 1)

                # attention; stage ctxT per head-pair, flush + AllGather.
                # ctx groups are interleaved right behind their diagonal
                # score tile, with Wo filler pacing the scalar-engine exps.
                nkb = NQS * sc + NQS  # key tiles visible to this chunk
                stage = None
                for h in range(HPC):
                    if h % 2 == 0:
                        stage = stgp.tile([P, 2, SCH], bf16, tag="stg")
                    pts = []
                    ctxns = []

                    def ctx_group(qh, pts=pts, ctxns=ctxns):
                        iqc = NQS * sc + qh
                        ps_c = psp.tile([P, SCH], f32, tag="a", bufs=3,
                                        name=f"psc{sc}_{qh}")
                        for kb in range(iqc + 1):
                            nc.tensor.matmul(
                                ps_c[:, 0:HD + 1],
                                pts[kb][:, qh * P:(qh + 1) * P],
                                vp_all[:, kb, :],
                                start=(kb == 0), stop=(kb == iqc))
                        rden = work.tile([P, 1], f32, tag="rden", bufs=4)
                        nc.vector.reciprocal(rden[:], ps_c[:, HD:HD + 1])
                        ctxn = work.tile([P, HD], bf16, tag="ctxn", bufs=4)
                        nc.vector.tensor_scalar_mul(ctxn[:], ps_c[:, 0:HD],
                                                    rden[:])
                        ctxns.append(ctxn)

                    for kb in range(nkb):
                        diag = kb - NQS * sc
                        off = diag * P if diag > 0 else 0
                        ps_s = psp.tile([P, SCH], f32, tag="a", bufs=3)
                        nc.tensor.matmul(ps_s[:, off:],
                                         kT_all[:, kb * P:(kb + 1) * P],
                                         qts[h][:, off:], start=True, stop=True)
                        pt = ptp.tile([P, SCH], bf16, tag="pt")
                        nc.scalar.activation(out=pt[:, off:], in_=ps_s[:, off:],
                                             func=Exp, scale=SCALE)
                        if 0 <= diag < NQS:
                            nc.vector.tensor_mul(
                                pt[:, diag * P:(diag + 1) * P],
                                pt[:, diag * P:(diag + 1) * P], trim[:])
                        pts.append(pt)
                        wos.take(1)
                        if diag >= 0:
                            ctx_group(diag)

                    for qh in range(NQS):
                        ps_t = psp.tile([P, P], bf16, tag="t", bufs=2)
                        nc.tensor.transpose(ps_t[:], ctxns[qh][:], ident[:])
                        nc.vector.tensor_copy(
                            stage[:, h % 2, qh * P:(qh + 1) * P], ps_t[:])
                    wos.take(2)

                    if sc == NSC - 1:
                        nc.scalar.dma_start(out=cc3_ins[h][:],
                                            in_=stage[:, h % 2, :])
                        nc.gpsimd.collective_compute(
                            "AllGather",
                            mybir.AluOpType.bypass,
                            replica_groups=[list(range(NC))],
                            ins=[cc3_ins[h][:]],
                            outs=[cc3_outs[h][:]],
                        )
                        wos.take(2)
                    elif h % 2 == 1:
                        pr = h // 2
                        nc.scalar.dma_start(
                            out=cc_ins[sc][pr][:].rearrange(
                                "(h p) c -> p h c", p=P),
                            in_=stage[:])
                        nc.gpsimd.collective_compute(
                            "AllGather",
                            mybir.AluOpType.bypass,
                            replica_groups=[list(range(NC))],
                            ins=[cc_ins[sc][pr][:]],
                            outs=[cc_outs[sc][pr][:]],
                        )
                        wos.take(2)

                # finish all Wo work whose AllGathers are long done, then
                # queue this chunk's split (consumable from next chunk)
                wos.drain()
                if sc < NSC - 1:
                    queue_wo_split(sc)

            # ================= tail: last split, single pass over 4 ob
            # accumulators (scores/ctx PSUM banks are free now), kt ordered
            # by head so tiles are consumed as their AllGathers land
            spl = NSC - 1
            accs3 = []
            for ob in range(4):
                tg = "o" if ob < 2 else "a"
                a = psp.tile([P, SCH], f32, tag=tg, bufs=(2 if tg == "o" else 3),
                             name=f"wo3_{ob}")
                accs3.append(a)
            kts3 = [HPC * r + h for h in range(HPC) for r in range(NC)]
            PF3 = 6
            ccts3 = {}

            def issue3(i):
                kt = kts3[i]
                h, r = kt % HPC, kt // HPC
                cct = ccp.tile([P, SCH], bf16, tag="ccp", name=f"cc3_{i}")
                nc.sync.dma_start(
                    out=cct[:],
                    in_=cc3_outs[h][:].rearrange("(t p) s -> p t s",
                                                 p=P)[:, r, :])
                ccts3[i] = cct

            for j in range(PF3):
                issue3(j)
            for i, kt in enumerate(kts3):
                if i + PF3 < len(kts3):
                    issue3(i + PF3)
                cct = ccts3.pop(i)
                for ob in range(4):
                    nc.tensor.matmul(
                        accs3[ob][:], wo_sb[:, kt, ob * P:(ob + 1) * P],
                        cct[:], start=(i == 0), stop=(i == len(kts3) - 1))
            for ob in range(4):
                o_sb = osbp.tile([P, SCH], bf16, tag="osb", name=f"osb3_{ob}")
                nc.vector.tensor_copy(o_sb[:], accs3[ob][:])
                nc.scalar.dma_start(
                    out=out[ob * P:(ob + 1) * P, spl * SCH:(spl + 1) * SCH],
                    in_=o_sb[:])

    return nc


_PROGRAM_CACHE = {}


def _get_program():
    if "nc" not in _PROGRAM_CACHE:
        nc = _build_program()
        _split_multi_waits(nc, maxw=1)
        _PROGRAM_CACHE["nc"] = nc
    return _PROGRAM_CACHE["nc"]


def _rope_tables_T():
    inv_freq = (1.0 / (10000.0 ** (np.arange(0, HD, 2, dtype=np.float32) / HD))
                ).astype(np.float32)
    ang = np.arange(S, dtype=np.float32)[:, None] * inv_freq[None, :]
    ang = np.concatenate([ang, ang], axis=-1)  # [S, HD]
    cosT = np.ascontiguousarray(np.cos(ang).T).astype(ml_dtypes.bfloat16)
    sinT = np.ascontiguousarray(np.sin(ang).T).astype(ml_dtypes.bfloat16)
    return cosT, sinT


def _prep_in_maps(x, Wq, Wk, Wv, Wo):
    bf = ml_dtypes.bfloat16
    x2d = np.asarray(x, np.float32).reshape(S, D).astype(bf)
    # [D, S] -> [P, NSC, KT, SCH]
    xT_dev = np.ascontiguousarray(
        x2d.T.reshape(KT, P, NSC, SCH).transpose(1, 2, 0, 3))
    cosT, sinT = _rope_tables_T()
    tri_np = (np.arange(P)[:, None] <= np.arange(P)[None, :]).astype(bf)

    def wtiles(Wslice, width):
        return np.ascontiguousarray(
            np.asarray(Wslice, np.float32).astype(bf)
            .reshape(KT, P, width).transpose(1, 0, 2))

    in_maps = []
    for c in range(NC):
        wq_c = np.asarray(Wq[:, c * OC:(c + 1) * OC], np.float32).astype(bf)
        wq_t = np.ascontiguousarray(
            wq_c.reshape(KT, P, HPC, HD).transpose(1, 2, 0, 3))
        in_maps.append({
            "xT": xT_dev,
            "wq": wq_t,
            "wk": wtiles(Wk[:, c * HD:(c + 1) * HD], HD),
            "wv": wtiles(Wv[:, c * HD:(c + 1) * HD], HD),
            "wo": wtiles(Wo[:, c * OC:(c + 1) * OC], OC),
            "cosT": cosT,
            "sinT": sinT,
            "tri": tri_np,
        })
    return in_maps


def _run(inputs, trace=False):
    nc = _get_program()
    in_maps = _prep_in_maps(inputs["x"], inputs["Wq"], inputs["Wk"],
                            inputs["Wv"], inputs["Wo"])
    res = run_bass_kernel_spmd(nc, in_maps, core_ids=list(range(NC)),
                               trace=trace)
    out = np.concatenate(
        [np.asarray(res.results[c]["out"]).astype(np.float32).T
         for c in range(NC)], axis=1)
    return out.reshape(B, S, D).astype(np.float32), res


def kernel(**inputs):
    out, _ = _run(inputs, trace=False)
    return out
